# revision 1
# baseline (speedup 1.0000x reference)
"""CFConv (SchNet continuous-filter conv) Trainium2 Bass kernel, 8-core SPMD.

Reference computation:
    f    = x @ W_in                        # (40000, 128)
    f_j  = f[idx_j]                        # (640000, 128) gather
    wf   = w_ij * f_j                      # elementwise
    conv = segment_sum(wf, seg_i, 40000)   # seg_i sorted
    out  = conv @ W_out + b_out

Sharding: seg_i is sorted, so atoms are sharded into 8 contiguous ranges of
5000 and each core gets the contiguous run of edges whose seg_i falls in its
range (found with searchsorted on the host).  No collective is needed: each
core owns its 5000 output rows.

Per core the edge run is re-bucketed by 128-atom sub-window of seg_i, each
sub-window padded to a fixed chunk capacity so all 8 cores run one identical
SPMD program.  Because dma_gather indices are int16, each sub-window's edges
are split by idx_j half (< 20000 vs >= 20000) into leading / trailing chunk
groups and gathered by two dma_gather calls (the second from an offset AP of
the f scratch).  On device:

  phase 1: f = x @ W_in into an HBM scratch (x passed pre-transposed so x
           tiles serve directly as matmul lhsT).
  phase 2: per sub-window: DMA the wf-ready w tile, dma_gather f[idx_j] rows,
           DVE multiply, build the one-hot segment matrix with an is_equal
           compare against an iota tile, and matmul-accumulate
           convT[feat, atom] in PSUM (contraction over the edge partition
           axis).  Per 1024-atom window: fac2out matmul with W_out + bias.
"""

import numpy as np

import concourse.bass as bass
import concourse.mybir as mybir
from concourse import bacc
from concourse.tile import TileContext

P = 128
NA = 40000          # atoms
NE = 640000         # edges
D = 128             # feature dim (FAN_IN == NFM == FAN_OUT)
HALF = NA // 2      # dma_gather int16 index limit workaround
NCORES = 8
APC = NA // NCORES  # atoms per core = 5000
WIN = 512           # atoms per PSUM window (1 bank)
SUB = 128           # atoms per sub-window (one matmul N slice)
NSW = (APC + SUB - 1) // SUB   # sub-windows per core = 40

F32 = mybir.dt.float32
I16 = mybir.dt.int16


def build_program(plan):
    """One SPMD program, identical across cores."""
    cap_lo, cap_hi, n16 = plan
    nc = bacc.Bacc(None, target_bir_lowering=False, debug=False)
    cap = cap_lo + cap_hi
    esw = cap * P
    icols = [n[0] // 16 + n[1] // 16 for n in n16]
    ioff = [0]
    for s in range(NSW):
        ioff.append(ioff[-1] + icols[s])

    xT_h = nc.dram_tensor("xT", [P, NA], F32, kind="ExternalInput")
    wdev_h = nc.dram_tensor("wdev", [NSW, P, esw], F32, kind="ExternalInput")
    segw_h = nc.dram_tensor("segw", [P, NSW * cap], F32, kind="ExternalInput")
    idx16_h = nc.dram_tensor("idx16", [P, ioff[-1]], I16, kind="ExternalInput")
    iota_h = nc.dram_tensor("iota", [P, esw], F32, kind="ExternalInput")
    win_h = nc.dram_tensor("Win", [P, P], F32, kind="ExternalInput")
    wout_h = nc.dram_tensor("Wout", [P, P], F32, kind="ExternalInput")
    bias_h = nc.dram_tensor("bias", [P, P], F32, kind="ExternalInput")
    out_h = nc.dram_tensor("out", [APC, D], F32, kind="ExternalOutput")
    # two tensors so lo-gathers only dep on the first half of phase 1
    flo_h = nc.dram_tensor("fscratch_lo", [HALF, D], F32, kind="Internal")
    fhi_h = nc.dram_tensor("fscratch_hi", [NA - HALF, D], F32, kind="Internal")

    with TileContext(nc) as tc:
        with tc.tile_pool(name="const", bufs=1) as const:
            win_t = const.tile([P, P], F32)
            nc.sync.dma_start(win_t[:], win_h[:, :])
            wout_t = const.tile([P, P], F32)
            nc.sync.dma_start(wout_t[:], wout_h[:, :])
            bias_t = const.tile([P, P], F32)
            nc.sync.dma_start(bias_t[:], bias_h[:, :])
            iota_t = const.tile([P, esw], F32)
            nc.sync.dma_start(iota_t[:], iota_h[:, :])
            segw_t = const.tile([P, NSW * cap], F32)
            nc.sync.dma_start(segw_t[:], segw_h[:, :])
            idx16_t = const.tile([P, ioff[-1]], I16)
            nc.sync.dma_start(idx16_t[:], idx16_h[:, :])

            # All pools open together: phase-2 tiles must NOT reuse
            # phase-1 SBUF addresses, else they inherit a WAR dep on all of
            # phase 1 (measured 98 us gpsimd stall).
            LOOK = 5  # lo-gather lookahead
            with (
                tc.tile_pool(name="xp", bufs=3) as xp,
                tc.tile_pool(name="fp", bufs=3) as fp,
                tc.tile_pool(name="ps1", bufs=2, space="PSUM") as ps1,
                tc.tile_pool(name="wp", bufs=3) as wp,
                tc.tile_pool(name="fjp", bufs=LOOK + 2) as fjp,
                tc.tile_pool(name="ohp", bufs=2) as ohp,
                tc.tile_pool(name="cvp", bufs=2) as cvp,
                tc.tile_pool(name="owp", bufs=2) as owp,
                tc.tile_pool(name="ps2", bufs=2, space="PSUM") as ps2,
                tc.tile_pool(name="ps3", bufs=2, space="PSUM") as ps3,
            ):
                # ---- phase 1: f = x @ W_in -> HBM scratch ----
                for half_h, h0 in ((flo_h, 0), (fhi_h, HALF)):
                    a0 = 0
                    hn = HALF if h0 == 0 else NA - HALF
                    while a0 < hn:
                        an = min(512, hn - a0)
                        xt = xp.tile([P, 512], F32)
                        nc.sync.dma_start(
                            xt[:, :an], xT_h[:, h0 + a0 : h0 + a0 + an]
                        )
                        fps = ps1.tile([P, 4, P], F32)
                        nt = (an + P - 1) // P
                        for i in range(nt):
                            m = min(P, an - i * P)
                            nc.tensor.matmul(
                                fps[:m, i, :],
                                lhsT=xt[:, i * P : i * P + m],
                                rhs=win_t[:],
                                start=True,
                                stop=True,
                            )
                        fsb = fp.tile([P, 4, P], F32)
                        if an % P == 0:
                            # tiled-contiguous f layout: HBM row a0 + p*4 + i
                            # holds atom a0 + i*128 + p (2 KB contiguous per
                            # partition); gather idxs are host-remapped to
                            # match.  The row-interleaved layout cost ~45%
                            # HBM write BW (512 B descs 64 KB apart).
                            nc.vector.tensor_copy(fsb[:, :nt, :], fps[:, :nt, :])
                            # scalar-engine HWDGE: keeps compute-gated f
                            # writes off the sync FIFO so x/w reads stream
                            # without head-of-line blocking
                            nc.scalar.dma_start(
                                half_h[a0 : a0 + an, :].rearrange(
                                    "(p i) e -> p i e", i=4
                                ),
                                fsb[:, :nt, :],
                            )
                        else:
                            nc.vector.tensor_copy(fsb[:an, 0, :], fps[:an, 0, :])
                            nc.scalar.dma_start(half_h[a0 : a0 + an, :], fsb[:an, 0, :])
                        a0 += an

                # ---- phase 2: gather, multiply, segment-sum, fac2out ----
                psT = None
                fj_q = {}

                def emit_lo(s):
                    # Static num_idxs is the 16-rounded max real count over
                    # cores (the Q7 scan cost tracks static num_idxs; runtime
                    # truncation buys nothing).  Pads within it gather row 0
                    # with w=0; the unwritten tail of the partial chunk is
                    # memzeroed on the idle ACT engine.  single_packet=False:
                    # >1008 idxs exceeds the 64-desc packet ceiling
                    # (HW-verified INTERNAL error otherwise).
                    nlo = n16[s][0]
                    clo = (nlo + P - 1) // P
                    fj = fjp.tile([P, cap, P], F32, tag="fj")
                    if nlo < cap_lo * P:
                        nc.scalar.memzero(fj[:, (nlo - 1) // P : cap_lo, :])
                    nc.gpsimd.dma_gather(
                        fj[:, 0:clo, :],
                        flo_h[:, :],
                        idx16_t[:, ioff[s] : ioff[s] + nlo // 16],
                        nlo,
                        nlo,
                        D,
                        single_packet=False,
                    )
                    fj_q[s] = fj

                for s in range(min(LOOK, NSW)):
                    emit_lo(s)
                for s in range(NSW):
                    w_i, sl = divmod(s, WIN // SUB)
                    wt = wp.tile([P, cap, P], F32)
                    nc.sync.dma_start(
                        wt[:], wdev_h[s].rearrange("p (c e) -> p c e", e=P)
                    )
                    fj = fj_q.pop(s)
                    nhi = n16[s][1]
                    chi = (nhi + P - 1) // P
                    if nhi < cap_hi * P:
                        nc.scalar.memzero(fj[:, cap_lo + (nhi - 1) // P : cap, :])
                    nc.gpsimd.dma_gather(
                        fj[:, cap_lo : cap_lo + chi, :],
                        fhi_h[:, :],
                        idx16_t[:, ioff[s] + n16[s][0] // 16 : ioff[s] + icols[s]],
                        nhi,
                        nhi,
                        D,
                        single_packet=False,
                    )
                    if s + LOOK < NSW:
                        emit_lo(s + LOOK)
                    nc.vector.tensor_mul(wt[:], wt[:], fj[:])
                    oh = ohp.tile([P, cap, P], F32)
                    nc.vector.tensor_tensor(
                        out=oh[:],
                        in0=segw_t[:, s * cap : (s + 1) * cap]
                        .unsqueeze(2)
                        .to_broadcast([P, cap, P]),
                        in1=iota_t[:].rearrange("p (c e) -> p c e", e=P),
                        op=mybir.AluOpType.is_equal,
                    )
                    if sl == 0:
                        psT = ps2.tile([P, WIN], F32)
                    for ch in range(cap):
                        nc.tensor.matmul(
                            psT[:, sl * SUB : (sl + 1) * SUB],
                            lhsT=wt[:, ch, :],
                            rhs=oh[:, ch, :],
                            start=(ch == 0),
                            stop=(ch == cap - 1),
                        )
                    if sl == WIN // SUB - 1 or s == NSW - 1:
                        wa0 = w_i * WIN
                        wan = min(WIN, APC - wa0)
                        cvt = cvp.tile([P, WIN], F32)
                        nc.vector.tensor_copy(cvt[:], psT[:])
                        ow = owp.tile([P, WIN // SUB, P], F32)
                        nblk = (wan + P - 1) // P
                        for b in range(nblk):
                            bm = min(P, wan - b * P)
                            ops3 = ps3.tile([P, P], F32)
                            nc.tensor.matmul(
                                ops3[:bm, :],
                                lhsT=cvt[:, b * P : b * P + bm],
                                rhs=wout_t[:],
                                start=True,
                                stop=True,
                            )
                            nc.vector.tensor_add(
                                ow[:bm, b, :], ops3[:bm, :], bias_t[:bm, :]
                            )
                        nfull = wan // P
                        if nfull:
                            nc.sync.dma_start(
                                out_h[wa0 : wa0 + nfull * P, :].rearrange(
                                    "(b p) e -> p b e", p=P
                                ),
                                ow[:, :nfull, :],
                            )
                        rem = wan - nfull * P
                        if rem:
                            nc.sync.dma_start(
                                out_h[wa0 + nfull * P : wa0 + wan, :],
                                ow[:rem, nfull, :],
                            )
    return nc


def _remap(j):
    """Atom index (within a 20000-row half) -> row in the tiled-contiguous
    f scratch layout written by phase 1 (identity for the partial tail)."""
    j = np.asarray(j)
    g, r = j // 512, j % 512
    return np.where(j >= (HALF // 512) * 512, j, g * 512 + (r % P) * 4 + r // P)


def _wrap_idx(idx):
    """idx [n] (n % 128 == 0) -> [128, n//16] int16 wrapped + replicated."""
    n = idx.shape[0]
    w = idx.reshape(n // 16, 16).T
    return np.tile(w, (8, 1)).astype(np.int16)


def prepare(inputs):
    """Host-side sharding: per-core padded edge buckets + gather indices."""
    x = np.ascontiguousarray(np.asarray(inputs["x"], dtype=np.float32))
    w_ij = np.ascontiguousarray(np.asarray(inputs["w_ij"], dtype=np.float32))
    seg_i = np.asarray(inputs["seg_i"]).astype(np.int64).ravel()
    idx_j = np.asarray(inputs["idx_j"]).astype(np.int64).ravel()
    W_in = np.ascontiguousarray(np.asarray(inputs["W_in"], dtype=np.float32))
    W_out = np.ascontiguousarray(np.asarray(inputs["W_out"], dtype=np.float32))
    b_out = np.asarray(inputs["b_out"], dtype=np.float32).ravel()

    # edge run boundaries for every 128-atom sub-window of every core
    bounds = []
    for c in range(NCORES):
        for s in range(NSW):
            bounds.append(c * APC + s * SUB)
    bounds.append(NA)
    edges = np.searchsorted(seg_i, np.asarray(bounds, dtype=np.int64))

    # per-sub-window lo/hi (by idx_j half) counts -> global chunk capacities
    nsw_tot = NCORES * NSW
    lo_masks = []
    n_lo = np.zeros(nsw_tot, dtype=np.int64)
    n_hi = np.zeros(nsw_tot, dtype=np.int64)
    for k in range(nsw_tot):
        lo, hi = edges[k], edges[k + 1]
        m = idx_j[lo:hi] < HALF
        lo_masks.append(m)
        n_lo[k] = int(m.sum())
        n_hi[k] = int((hi - lo) - n_lo[k])
    cap_lo = max(1, int(-(-n_lo.max() // P)))
    cap_hi = max(1, int(-(-n_hi.max() // P)))
    cap = cap_lo + cap_hi
    esw = cap * P
    # per-(s,half) static gather sizes: 16-rounded max real count over cores
    n_lo2 = n_lo.reshape(NCORES, NSW)
    n_hi2 = n_hi.reshape(NCORES, NSW)
    n16 = []
    for s in range(NSW):
        n16.append(
            (
                max(16, int(-(-n_lo2[:, s].max() // 16)) * 16),
                max(16, int(-(-n_hi2[:, s].max() // 16)) * 16),
            )
        )
    icols = [n[0] // 16 + n[1] // 16 for n in n16]
    ntot = sum(icols)

    iota_t = np.tile(np.arange(P, dtype=np.float32), (P, cap))
    bias_t = np.tile(b_out[None, :], (P, 1)).astype(np.float32)
    xT = np.ascontiguousarray(x.T)

    in_maps = []
    for c in range(NCORES):
        wdev = np.zeros((NSW, P, esw), dtype=np.float32)
        segw = np.zeros((P, NSW * cap), dtype=np.float32)
        idx16 = np.zeros((P, ntot), dtype=np.int16)
        for s in range(NSW):
            k = c * NSW + s
            lo, hi = edges[k], edges[k + 1]
            m = lo_masks[k]
            e_idx = idx_j[lo:hi]
            e_seg = (seg_i[lo:hi] - (c * APC + s * SUB)).astype(np.float32)
            e_w = w_ij[lo:hi]
            nl = int(n_lo[k])
            nh = int(n_hi[k])

            wpad = np.zeros((esw, D), dtype=np.float32)
            spad = np.zeros(esw, dtype=np.float32)
            ilo = np.zeros(n16[s][0], dtype=np.int16)
            ihi = np.zeros(n16[s][1], dtype=np.int16)

            wpad[:nl] = e_w[m]
            spad[:nl] = e_seg[m]
            ilo[:nl] = _remap(e_idx[m]).astype(np.int16)
            base = cap_lo * P
            wpad[base : base + nh] = e_w[~m]
            spad[base : base + nh] = e_seg[~m]
            ihi[:nh] = _remap(e_idx[~m] - HALF).astype(np.int16)

            wdev[s] = wpad.reshape(cap, P, D).transpose(1, 0, 2).reshape(P, esw)
            segw[:, s * cap : (s + 1) * cap] = spad.reshape(cap, P).T
            io = sum(icols[:s])
            idx16[:, io : io + n16[s][0] // 16] = _wrap_idx(ilo)
            idx16[:, io + n16[s][0] // 16 : io + icols[s]] = _wrap_idx(ihi)
        in_maps.append(
            {
                "xT": xT,
                "wdev": wdev,
                "segw": segw,
                "idx16": idx16,
                "iota": iota_t,
                "Win": W_in,
                "Wout": W_out,
                "bias": bias_t,
            }
        )
    return (cap_lo, cap_hi, n16), in_maps


def kernel(**inputs) -> np.ndarray:
    from concourse.bass_utils import run_bass_kernel_spmd

    plan, in_maps = prepare(inputs)
    nc = build_program(plan)
    nc.finalize()
    res = run_bass_kernel_spmd(nc, in_maps, core_ids=list(range(NCORES)))
    return np.concatenate([r["out"] for r in res.results], axis=0)



# revision 10
# speedup vs baseline: 1.6101x; 1.6101x over previous
"""CFConv (SchNet continuous-filter conv) Trainium2 Bass kernel, 8-core SPMD.

Reference computation:
    f    = x @ W_in                        # (40000, 128)
    f_j  = f[idx_j]                        # (640000, 128) gather
    wf   = w_ij * f_j                      # elementwise
    conv = segment_sum(wf, seg_i, 40000)   # seg_i sorted
    out  = conv @ W_out + b_out

Sharding: seg_i is sorted, so atoms are sharded into 8 contiguous ranges of
5000 and each core gets the contiguous run of edges whose seg_i falls in its
range (found with searchsorted on the host).  No collective is needed: each
core owns its 5000 output rows.

Per core the edge run is re-bucketed by 128-atom sub-window of seg_i, each
sub-window padded to a fixed chunk capacity so all 8 cores run one identical
SPMD program.  Because dma_gather indices are int16, each sub-window's edges
are split by idx_j half (< 20000 vs >= 20000) into leading / trailing chunk
groups and gathered by two dma_gather calls (the second from an offset AP of
the f scratch).  On device:

  phase 1: f = x @ W_in into an HBM scratch (x passed pre-transposed so x
           tiles serve directly as matmul lhsT).
  phase 2: per sub-window: DMA the wf-ready w tile, dma_gather f[idx_j] rows,
           DVE multiply, build the one-hot segment matrix with an is_equal
           compare against an iota tile, and matmul-accumulate
           convT[feat, atom] in PSUM (contraction over the edge partition
           axis).  Per 1024-atom window: fac2out matmul with W_out + bias.
"""

import numpy as np

import concourse.bass as bass
import concourse.mybir as mybir
from concourse import bacc
from concourse.tile import TileContext

P = 128
NA = 40000          # atoms
NE = 640000         # edges
D = 128             # feature dim (FAN_IN == NFM == FAN_OUT)
HALF = NA // 2      # dma_gather int16 index limit workaround
NCORES = 8
APC = NA // NCORES  # atoms per core = 5000
WIN = 512           # atoms per PSUM window (1 bank)
SUB = 128           # atoms per sub-window (one matmul N slice)
NSW = (APC + SUB - 1) // SUB   # sub-windows per core = 40

F32 = mybir.dt.float32
BF16 = mybir.dt.bfloat16
I16 = mybir.dt.int16


def build_program(plan):
    """One SPMD program, identical across cores."""
    cap_lo, cap_hi, n16 = plan
    nc = bacc.Bacc(
        None, target_bir_lowering=False, debug=False, num_swdge_queues=4
    )
    cap = cap_lo + cap_hi
    esw = cap * P
    icols = [n[0] // 16 + n[1] // 16 for n in n16]
    ioff = [0]
    for s in range(NSW):
        ioff.append(ioff[-1] + icols[s])

    xT_h = nc.dram_tensor("xT", [P, NA], BF16, kind="ExternalInput")
    wdev_h = nc.dram_tensor("wdev", [NSW, P, esw], BF16, kind="ExternalInput")
    segw_h = nc.dram_tensor("segw", [P, NSW * cap], BF16, kind="ExternalInput")
    idx16_h = nc.dram_tensor("idx16", [P, ioff[-1]], I16, kind="ExternalInput")
    iota_h = nc.dram_tensor("iota", [P, esw], BF16, kind="ExternalInput")
    win_h = nc.dram_tensor("Win", [P, P], BF16, kind="ExternalInput")
    wout_h = nc.dram_tensor("Wout", [P, P], BF16, kind="ExternalInput")
    bias_h = nc.dram_tensor("bias", [P, P], F32, kind="ExternalInput")
    out_h = nc.dram_tensor("out", [APC, D], F32, kind="ExternalOutput")
    # two tensors so lo-gathers only dep on the first half of phase 1
    flo_h = nc.dram_tensor("fscratch_lo", [HALF, D], BF16, kind="Internal")
    fhi_h = nc.dram_tensor("fscratch_hi", [NA - HALF, D], BF16, kind="Internal")

    with TileContext(nc) as tc:
        with tc.tile_pool(name="const", bufs=1) as const:
            win_t = const.tile([P, P], BF16)
            nc.sync.dma_start(win_t[:], win_h[:, :])
            wout_t = const.tile([P, P], BF16)
            nc.sync.dma_start(wout_t[:], wout_h[:, :])
            bias_t = const.tile([P, P], F32)
            nc.sync.dma_start(bias_t[:], bias_h[:, :])
            iota_t = const.tile([P, esw], BF16)
            nc.sync.dma_start(iota_t[:], iota_h[:, :])
            segw_t = const.tile([P, NSW * cap], BF16)
            nc.sync.dma_start(segw_t[:], segw_h[:, :])
            idx16_t = const.tile([P, ioff[-1]], I16)
            nc.sync.dma_start(idx16_t[:], idx16_h[:, :])

            # All pools open together: phase-2 tiles must NOT reuse
            # phase-1 SBUF addresses, else they inherit a WAR dep on all of
            # phase 1 (measured 98 us gpsimd stall).
            LOOK = 5  # lo-gather lookahead
            with (
                tc.tile_pool(name="xp", bufs=3) as xp,
                tc.tile_pool(name="fp", bufs=3) as fp,
                tc.tile_pool(name="ps1", bufs=2, space="PSUM") as ps1,
                tc.tile_pool(name="wp", bufs=3) as wp,
                tc.tile_pool(name="fjp", bufs=LOOK + 2) as fjp,
                tc.tile_pool(name="ohp", bufs=2) as ohp,
                tc.tile_pool(name="cvp", bufs=2) as cvp,
                tc.tile_pool(name="owp", bufs=2) as owp,
                tc.tile_pool(name="ps2", bufs=2, space="PSUM") as ps2,
                tc.tile_pool(name="ps3", bufs=2, space="PSUM") as ps3,
            ):
                # ---- phase 1: f = x @ W_in -> HBM scratch ----
                for half_h, h0 in ((flo_h, 0), (fhi_h, HALF)):
                    a0 = 0
                    hn = HALF if h0 == 0 else NA - HALF
                    while a0 < hn:
                        an = min(512, hn - a0)
                        xt = xp.tile([P, 512], BF16)
                        nc.sync.dma_start(
                            xt[:, :an], xT_h[:, h0 + a0 : h0 + a0 + an]
                        )
                        fps = ps1.tile([P, 4, P], F32)
                        nt = (an + P - 1) // P
                        for i in range(nt):
                            m = min(P, an - i * P)
                            nc.tensor.matmul(
                                fps[:m, i, :],
                                lhsT=xt[:, i * P : i * P + m],
                                rhs=win_t[:],
                                start=True,
                                stop=True,
                            )
                        fsb = fp.tile([P, 4, P], BF16)
                        if an % P == 0:
                            # tiled-contiguous f layout: HBM row a0 + p*4 + i
                            # holds atom a0 + i*128 + p (2 KB contiguous per
                            # partition); gather idxs are host-remapped to
                            # match.  The row-interleaved layout cost ~45%
                            # HBM write BW (512 B descs 64 KB apart).
                            nc.vector.tensor_copy(fsb[:, :nt, :], fps[:, :nt, :])
                            # scalar-engine HWDGE: keeps compute-gated f
                            # writes off the sync FIFO so x/w reads stream
                            # without head-of-line blocking
                            nc.scalar.dma_start(
                                half_h[a0 : a0 + an, :].rearrange(
                                    "(p i) e -> p i e", i=4
                                ),
                                fsb[:, :nt, :],
                            )
                        else:
                            nc.vector.tensor_copy(fsb[:an, 0, :], fps[:an, 0, :])
                            nc.scalar.dma_start(half_h[a0 : a0 + an, :], fsb[:an, 0, :])
                        a0 += an

                # ---- phase 2: gather, multiply, segment-sum, fac2out ----
                psT = None
                fj_q = {}

                def emit_lo(s):
                    # Static num_idxs is the 16-rounded max real count over
                    # cores (the Q7 scan cost tracks static num_idxs; runtime
                    # truncation buys nothing).  Pads within it gather row 0
                    # with w=0; the unwritten tail of the partial chunk is
                    # memzeroed on the idle ACT engine.  single_packet=False:
                    # >1008 idxs exceeds the 64-desc packet ceiling
                    # (HW-verified INTERNAL error otherwise).
                    nlo = n16[s][0]
                    clo = (nlo + P - 1) // P
                    fj = fjp.tile([P, cap, P], BF16, tag="fj")
                    if nlo < cap_lo * P:
                        nc.scalar.memzero(fj[:, (nlo - 1) // P : cap_lo, :])
                    nc.gpsimd.dma_gather(
                        fj[:, 0:clo, :],
                        flo_h[:, :],
                        idx16_t[:, ioff[s] : ioff[s] + nlo // 16],
                        nlo,
                        nlo,
                        D,
                        single_packet=False,
                        queue_num=(2 * s) % 4,
                    )
                    fj_q[s] = fj

                for s in range(min(LOOK, NSW)):
                    emit_lo(s)
                for s in range(NSW):
                    w_i, sl = divmod(s, WIN // SUB)
                    wt = wp.tile([P, cap, P], F32)
                    nc.sync.dma_start(
                        wt[:], wdev_h[s].rearrange("p (c e) -> p c e", e=P)
                    )
                    fj = fj_q.pop(s)
                    nhi = n16[s][1]
                    chi = (nhi + P - 1) // P
                    if nhi < cap_hi * P:
                        nc.scalar.memzero(fj[:, cap_lo + (nhi - 1) // P : cap, :])
                    nc.gpsimd.dma_gather(
                        fj[:, cap_lo : cap_lo + chi, :],
                        fhi_h[:, :],
                        idx16_t[:, ioff[s] + n16[s][0] // 16 : ioff[s] + icols[s]],
                        nhi,
                        nhi,
                        D,
                        single_packet=False,
                        queue_num=(2 * s + 1) % 4,
                    )
                    if s + LOOK < NSW:
                        emit_lo(s + LOOK)
                    nc.vector.tensor_mul(wt[:], wt[:], fj[:])
                    oh = ohp.tile([P, cap, P], F32)
                    nc.vector.tensor_tensor(
                        out=oh[:],
                        in0=segw_t[:, s * cap : (s + 1) * cap]
                        .unsqueeze(2)
                        .to_broadcast([P, cap, P]),
                        in1=iota_t[:].rearrange("p (c e) -> p c e", e=P),
                        op=mybir.AluOpType.is_equal,
                    )
                    if sl == 0:
                        psT = ps2.tile([P, WIN], F32)
                    for ch in range(cap):
                        nc.tensor.matmul(
                            psT[:, sl * SUB : (sl + 1) * SUB],
                            lhsT=wt[:, ch, :],
                            rhs=oh[:, ch, :],
                            start=(ch == 0),
                            stop=(ch == cap - 1),
                        )
                    if sl == WIN // SUB - 1 or s == NSW - 1:
                        wa0 = w_i * WIN
                        wan = min(WIN, APC - wa0)
                        cvt = cvp.tile([P, WIN], F32)
                        nc.vector.tensor_copy(cvt[:], psT[:])
                        ow = owp.tile([P, WIN // SUB, P], F32)
                        nblk = (wan + P - 1) // P
                        for b in range(nblk):
                            bm = min(P, wan - b * P)
                            ops3 = ps3.tile([P, P], F32)
                            nc.tensor.matmul(
                                ops3[:bm, :],
                                lhsT=cvt[:, b * P : b * P + bm],
                                rhs=wout_t[:],
                                start=True,
                                stop=True,
                            )
                            nc.vector.tensor_add(
                                ow[:bm, b, :], ops3[:bm, :], bias_t[:bm, :]
                            )
                        nfull = wan // P
                        if nfull:
                            nc.sync.dma_start(
                                out_h[wa0 : wa0 + nfull * P, :].rearrange(
                                    "(b p) e -> p b e", p=P
                                ),
                                ow[:, :nfull, :],
                            )
                        rem = wan - nfull * P
                        if rem:
                            nc.sync.dma_start(
                                out_h[wa0 + nfull * P : wa0 + wan, :],
                                ow[:rem, nfull, :],
                            )
    return nc


def _remap(j):
    """Atom index (within a 20000-row half) -> row in the tiled-contiguous
    f scratch layout written by phase 1 (identity for the partial tail)."""
    j = np.asarray(j)
    g, r = j // 512, j % 512
    return np.where(j >= (HALF // 512) * 512, j, g * 512 + (r % P) * 4 + r // P)


def _wrap_idx(idx):
    """idx [n] (n % 128 == 0) -> [128, n//16] int16 wrapped + replicated."""
    n = idx.shape[0]
    w = idx.reshape(n // 16, 16).T
    return np.tile(w, (8, 1)).astype(np.int16)


def prepare(inputs):
    """Host-side sharding: per-core padded edge buckets + gather indices."""
    x = np.ascontiguousarray(np.asarray(inputs["x"], dtype=np.float32))
    w_ij = np.ascontiguousarray(np.asarray(inputs["w_ij"], dtype=np.float32))
    seg_i = np.asarray(inputs["seg_i"]).astype(np.int64).ravel()
    idx_j = np.asarray(inputs["idx_j"]).astype(np.int64).ravel()
    W_in = np.ascontiguousarray(np.asarray(inputs["W_in"], dtype=np.float32))
    W_out = np.ascontiguousarray(np.asarray(inputs["W_out"], dtype=np.float32))
    b_out = np.asarray(inputs["b_out"], dtype=np.float32).ravel()

    # edge run boundaries for every 128-atom sub-window of every core
    bounds = []
    for c in range(NCORES):
        for s in range(NSW):
            bounds.append(c * APC + s * SUB)
    bounds.append(NA)
    edges = np.searchsorted(seg_i, np.asarray(bounds, dtype=np.int64))

    # per-sub-window lo/hi (by idx_j half) counts -> global chunk capacities
    nsw_tot = NCORES * NSW
    lo_masks = []
    n_lo = np.zeros(nsw_tot, dtype=np.int64)
    n_hi = np.zeros(nsw_tot, dtype=np.int64)
    for k in range(nsw_tot):
        lo, hi = edges[k], edges[k + 1]
        m = idx_j[lo:hi] < HALF
        lo_masks.append(m)
        n_lo[k] = int(m.sum())
        n_hi[k] = int((hi - lo) - n_lo[k])
    cap_lo = max(1, int(-(-n_lo.max() // P)))
    cap_hi = max(1, int(-(-n_hi.max() // P)))
    cap = cap_lo + cap_hi
    esw = cap * P
    # per-(s,half) static gather sizes: 16-rounded max real count over cores
    n_lo2 = n_lo.reshape(NCORES, NSW)
    n_hi2 = n_hi.reshape(NCORES, NSW)
    n16 = []
    for s in range(NSW):
        n16.append(
            (
                max(16, int(-(-n_lo2[:, s].max() // 16)) * 16),
                max(16, int(-(-n_hi2[:, s].max() // 16)) * 16),
            )
        )
    icols = [n[0] // 16 + n[1] // 16 for n in n16]
    ntot = sum(icols)

    iota_t = np.tile(np.arange(P, dtype=np.float32), (P, cap))
    bias_t = np.tile(b_out[None, :], (P, 1)).astype(np.float32)
    xT = np.ascontiguousarray(x.T)

    in_maps = []
    for c in range(NCORES):
        wdev = np.zeros((NSW, P, esw), dtype=np.float32)
        segw = np.zeros((P, NSW * cap), dtype=np.float32)
        idx16 = np.zeros((P, ntot), dtype=np.int16)
        for s in range(NSW):
            k = c * NSW + s
            lo, hi = edges[k], edges[k + 1]
            m = lo_masks[k]
            e_idx = idx_j[lo:hi]
            e_seg = (seg_i[lo:hi] - (c * APC + s * SUB)).astype(np.float32)
            e_w = w_ij[lo:hi]
            nl = int(n_lo[k])
            nh = int(n_hi[k])

            wpad = np.zeros((esw, D), dtype=np.float32)
            spad = np.zeros(esw, dtype=np.float32)
            ilo = np.zeros(n16[s][0], dtype=np.int16)
            ihi = np.zeros(n16[s][1], dtype=np.int16)

            wpad[:nl] = e_w[m]
            spad[:nl] = e_seg[m]
            ilo[:nl] = _remap(e_idx[m]).astype(np.int16)
            base = cap_lo * P
            wpad[base : base + nh] = e_w[~m]
            spad[base : base + nh] = e_seg[~m]
            ihi[:nh] = _remap(e_idx[~m] - HALF).astype(np.int16)

            wdev[s] = wpad.reshape(cap, P, D).transpose(1, 0, 2).reshape(P, esw)
            segw[:, s * cap : (s + 1) * cap] = spad.reshape(cap, P).T
            io = sum(icols[:s])
            idx16[:, io : io + n16[s][0] // 16] = _wrap_idx(ilo)
            idx16[:, io + n16[s][0] // 16 : io + icols[s]] = _wrap_idx(ihi)
        in_maps.append(
            {
                "xT": xT,
                "wdev": wdev,
                "segw": segw,
                "idx16": idx16,
                "iota": iota_t,
                "Win": W_in,
                "Wout": W_out,
                "bias": bias_t,
            }
        )
    return (cap_lo, cap_hi, n16), in_maps


def kernel(**inputs) -> np.ndarray:
    from concourse.bass_utils import run_bass_kernel_spmd

    plan, in_maps = prepare(inputs)
    nc = build_program(plan)
    nc.finalize()
    res = run_bass_kernel_spmd(nc, in_maps, core_ids=list(range(NCORES)))
    return np.concatenate([r["out"] for r in res.results], axis=0)



# revision 16
# speedup vs baseline: 2.3910x; 1.4850x over previous
"""CFConv (SchNet continuous-filter conv) Trainium2 Bass kernel, 8-core SPMD.

Reference computation:
    f    = x @ W_in                        # (40000, 128)
    f_j  = f[idx_j]                        # (640000, 128) gather
    wf   = w_ij * f_j                      # elementwise
    conv = segment_sum(wf, seg_i, 40000)   # seg_i sorted
    out  = conv @ W_out + b_out

Sharding: seg_i is sorted, so atoms are sharded into 8 contiguous ranges of
5000 and each core gets the contiguous run of edges whose seg_i falls in its
range (found with searchsorted on the host).  No collective is needed: each
core owns its 5000 output rows.

Per core the edge run is re-bucketed by 128-atom sub-window of seg_i, each
sub-window padded to a fixed chunk capacity so all 8 cores run one identical
SPMD program.  Because dma_gather indices are int16, each sub-window's edges
are split by idx_j half (< 20000 vs >= 20000) into leading / trailing chunk
groups and gathered by two dma_gather calls (the second from an offset AP of
the f scratch).  On device:

  phase 1: f = x @ W_in into an HBM scratch (x passed pre-transposed so x
           tiles serve directly as matmul lhsT).
  phase 2: per sub-window: DMA the wf-ready w tile, dma_gather f[idx_j] rows,
           DVE multiply, build the one-hot segment matrix with an is_equal
           compare against an iota tile, and matmul-accumulate
           convT[feat, atom] in PSUM (contraction over the edge partition
           axis).  Per 1024-atom window: fac2out matmul with W_out + bias.
"""

import numpy as np
import ml_dtypes

import concourse.bass as bass
import concourse.mybir as mybir
from concourse import bacc
from concourse.tile import TileContext

P = 128
NA = 40000          # atoms
NE = 640000         # edges
D = 128             # feature dim (FAN_IN == NFM == FAN_OUT)
HALF = NA // 2      # dma_gather int16 index limit workaround
NCORES = 8
APC = NA // NCORES  # atoms per core = 5000
WIN = 512           # atoms per PSUM window (1 bank)
SUB = 128           # atoms per sub-window (one matmul N slice)
NSW = (APC + SUB - 1) // SUB   # sub-windows per core = 40

F32 = mybir.dt.float32
BF16 = mybir.dt.bfloat16
I16 = mybir.dt.int16


def build_program(plan):
    """One SPMD program, identical across cores."""
    cap_lo, cap_hi, n16 = plan
    nc = bacc.Bacc(
        None, target_bir_lowering=False, debug=False, num_swdge_queues=4
    )
    cap = cap_lo + cap_hi
    esw = cap * P
    icols = [n[0] // 16 + n[1] // 16 for n in n16]
    ioff = [0]
    for s in range(NSW):
        ioff.append(ioff[-1] + icols[s])

    xT_h = nc.dram_tensor("xT", [P, NA], BF16, kind="ExternalInput")
    wdev_h = nc.dram_tensor("wdev", [NSW, P, esw], BF16, kind="ExternalInput")
    segw_h = nc.dram_tensor("segw", [P, NSW * cap], BF16, kind="ExternalInput")
    idx16_h = nc.dram_tensor("idx16", [P, ioff[-1]], I16, kind="ExternalInput")
    iota_h = nc.dram_tensor("iota", [P, esw], BF16, kind="ExternalInput")
    win_h = nc.dram_tensor("Win", [P, P], BF16, kind="ExternalInput")
    wout_h = nc.dram_tensor("Wout", [P, P], BF16, kind="ExternalInput")
    bias_h = nc.dram_tensor("bias", [P, P], F32, kind="ExternalInput")
    out_h = nc.dram_tensor("out", [APC, D], F32, kind="ExternalOutput")
    # two tensors so lo-gathers only dep on the first half of phase 1
    flo_h = nc.dram_tensor("fscratch_lo", [HALF, D], BF16, kind="Internal")
    fhi_h = nc.dram_tensor("fscratch_hi", [NA - HALF, D], BF16, kind="Internal")

    with TileContext(nc) as tc:
        with tc.tile_pool(name="const", bufs=1) as const:
            win_t = const.tile([P, P], BF16)
            nc.sync.dma_start(win_t[:], win_h[:, :])
            wout_t = const.tile([P, P], BF16)
            nc.sync.dma_start(wout_t[:], wout_h[:, :])
            bias_t = const.tile([P, P], F32)
            nc.sync.dma_start(bias_t[:], bias_h[:, :])
            iota_t = const.tile([P, esw], BF16)
            nc.sync.dma_start(iota_t[:], iota_h[:, :])
            segw_t = const.tile([P, NSW * cap], BF16)
            nc.sync.dma_start(segw_t[:], segw_h[:, :])
            idx16_t = const.tile([P, ioff[-1]], I16)
            nc.sync.dma_start(idx16_t[:], idx16_h[:, :])

            # All pools open together: phase-2 tiles must NOT reuse
            # phase-1 SBUF addresses, else they inherit a WAR dep on all of
            # phase 1 (measured 98 us gpsimd stall).
            LOOK = 5  # lo-gather lookahead
            with (
                tc.tile_pool(name="xp", bufs=3) as xp,
                tc.tile_pool(name="fp", bufs=3) as fp,
                tc.tile_pool(name="ps1", bufs=2, space="PSUM") as ps1,
                tc.tile_pool(name="wp", bufs=3) as wp,
                tc.tile_pool(name="fjp", bufs=LOOK + 2) as fjp,
                tc.tile_pool(name="ohp", bufs=2) as ohp,
                tc.tile_pool(name="cvp", bufs=2) as cvp,
                tc.tile_pool(name="owp", bufs=2) as owp,
                tc.tile_pool(name="ps2", bufs=2, space="PSUM") as ps2,
                tc.tile_pool(name="ps3", bufs=2, space="PSUM") as ps3,
            ):
                # ---- phase 1: f = x @ W_in -> HBM scratch ----
                for half_h, h0 in ((flo_h, 0), (fhi_h, HALF)):
                    a0 = 0
                    hn = HALF if h0 == 0 else NA - HALF
                    while a0 < hn:
                        an = min(512, hn - a0)
                        xt = xp.tile([P, 512], BF16)
                        nc.sync.dma_start(
                            xt[:, :an], xT_h[:, h0 + a0 : h0 + a0 + an]
                        )
                        fps = ps1.tile([P, 4, P], F32)
                        nt = (an + P - 1) // P
                        for i in range(nt):
                            m = min(P, an - i * P)
                            nc.tensor.matmul(
                                fps[:m, i, :],
                                lhsT=xt[:, i * P : i * P + m],
                                rhs=win_t[:],
                                start=True,
                                stop=True,
                            )
                        fsb = fp.tile([P, 4, P], BF16)
                        if an % P == 0:
                            # tiled-contiguous f layout: HBM row a0 + p*4 + i
                            # holds atom a0 + i*128 + p (2 KB contiguous per
                            # partition); gather idxs are host-remapped to
                            # match.  The row-interleaved layout cost ~45%
                            # HBM write BW (512 B descs 64 KB apart).
                            nc.vector.tensor_copy(fsb[:, :nt, :], fps[:, :nt, :])
                            # scalar-engine HWDGE: keeps compute-gated f
                            # writes off the sync FIFO so x/w reads stream
                            # without head-of-line blocking
                            nc.scalar.dma_start(
                                half_h[a0 : a0 + an, :].rearrange(
                                    "(p i) e -> p i e", i=4
                                ),
                                fsb[:, :nt, :],
                            )
                        else:
                            nc.vector.tensor_copy(fsb[:an, 0, :], fps[:an, 0, :])
                            nc.scalar.dma_start(half_h[a0 : a0 + an, :], fsb[:an, 0, :])
                        a0 += an

                # ---- phase 2: gather, multiply, segment-sum, fac2out ----
                psT = None
                fj_q = {}

                def emit_lo(s):
                    # Static num_idxs is the 16-rounded max real count over
                    # cores (the Q7 scan cost tracks static num_idxs; runtime
                    # truncation buys nothing).  Pads within it gather row 0
                    # with w=0; the unwritten tail of the partial chunk is
                    # memzeroed on the idle ACT engine.  single_packet=False:
                    # >1008 idxs exceeds the 64-desc packet ceiling
                    # (HW-verified INTERNAL error otherwise).
                    nlo = n16[s][0]
                    clo = (nlo + P - 1) // P
                    fj = fjp.tile([P, cap, P], BF16, tag="fj")
                    if nlo < cap_lo * P:
                        nc.scalar.memzero(fj[:, (nlo - 1) // P : cap_lo, :])
                    nc.gpsimd.dma_gather(
                        fj[:, 0:clo, :],
                        flo_h[:, :],
                        idx16_t[:, ioff[s] : ioff[s] + nlo // 16],
                        nlo,
                        nlo,
                        D,
                        single_packet=False,
                        queue_num=(2 * s) % 4,
                    )
                    fj_q[s] = fj

                for s in range(min(LOOK, NSW)):
                    emit_lo(s)
                for s in range(NSW):
                    w_i, sl = divmod(s, WIN // SUB)
                    wt = wp.tile([P, cap, P], BF16)
                    nc.sync.dma_start(
                        wt[:], wdev_h[s].rearrange("p (c e) -> p c e", e=P)
                    )
                    fj = fj_q.pop(s)
                    nhi = n16[s][1]
                    chi = (nhi + P - 1) // P
                    if nhi < cap_hi * P:
                        nc.scalar.memzero(fj[:, cap_lo + (nhi - 1) // P : cap, :])
                    nc.gpsimd.dma_gather(
                        fj[:, cap_lo : cap_lo + chi, :],
                        fhi_h[:, :],
                        idx16_t[:, ioff[s] + n16[s][0] // 16 : ioff[s] + icols[s]],
                        nhi,
                        nhi,
                        D,
                        single_packet=False,
                        queue_num=(2 * s + 1) % 4,
                    )
                    if s + LOOK < NSW:
                        emit_lo(s + LOOK)
                    nc.vector.tensor_mul(wt[:], wt[:], fj[:])
                    oh = ohp.tile([P, cap, P], BF16)
                    nc.vector.tensor_tensor(
                        out=oh[:],
                        in0=segw_t[:, s * cap : (s + 1) * cap]
                        .unsqueeze(2)
                        .to_broadcast([P, cap, P]),
                        in1=iota_t[:].rearrange("p (c e) -> p c e", e=P),
                        op=mybir.AluOpType.is_equal,
                    )
                    if sl == 0:
                        psT = ps2.tile([P, WIN], F32)
                    for ch in range(cap):
                        nc.tensor.matmul(
                            psT[:, sl * SUB : (sl + 1) * SUB],
                            lhsT=wt[:, ch, :],
                            rhs=oh[:, ch, :],
                            start=(ch == 0),
                            stop=(ch == cap - 1),
                        )
                    if sl == WIN // SUB - 1 or s == NSW - 1:
                        wa0 = w_i * WIN
                        wan = min(WIN, APC - wa0)
                        cvt = cvp.tile([P, WIN], BF16)
                        nc.vector.tensor_copy(cvt[:], psT[:])
                        ow = owp.tile([P, WIN // SUB, P], F32)
                        nblk = (wan + P - 1) // P
                        for b in range(nblk):
                            bm = min(P, wan - b * P)
                            ops3 = ps3.tile([P, P], F32)
                            nc.tensor.matmul(
                                ops3[:bm, :],
                                lhsT=cvt[:, b * P : b * P + bm],
                                rhs=wout_t[:],
                                start=True,
                                stop=True,
                            )
                            nc.vector.tensor_add(
                                ow[:bm, b, :], ops3[:bm, :], bias_t[:bm, :]
                            )
                        nfull = wan // P
                        if nfull:
                            nc.sync.dma_start(
                                out_h[wa0 : wa0 + nfull * P, :].rearrange(
                                    "(b p) e -> p b e", p=P
                                ),
                                ow[:, :nfull, :],
                            )
                        rem = wan - nfull * P
                        if rem:
                            nc.sync.dma_start(
                                out_h[wa0 + nfull * P : wa0 + wan, :],
                                ow[:rem, nfull, :],
                            )
    return nc


def _remap(j):
    """Atom index (within a 20000-row half) -> row in the tiled-contiguous
    f scratch layout written by phase 1 (identity for the partial tail)."""
    j = np.asarray(j)
    g, r = j // 512, j % 512
    return np.where(j >= (HALF // 512) * 512, j, g * 512 + (r % P) * 4 + r // P)


def _wrap_idx(idx):
    """idx [n] (n % 128 == 0) -> [128, n//16] int16 wrapped + replicated."""
    n = idx.shape[0]
    w = idx.reshape(n // 16, 16).T
    return np.tile(w, (8, 1)).astype(np.int16)


def prepare(inputs):
    """Host-side sharding: per-core padded edge buckets + gather indices."""
    x = np.ascontiguousarray(np.asarray(inputs["x"], dtype=np.float32))
    w_ij = np.ascontiguousarray(np.asarray(inputs["w_ij"], dtype=np.float32))
    seg_i = np.asarray(inputs["seg_i"]).astype(np.int64).ravel()
    idx_j = np.asarray(inputs["idx_j"]).astype(np.int64).ravel()
    W_in = np.ascontiguousarray(np.asarray(inputs["W_in"], dtype=np.float32))
    W_out = np.ascontiguousarray(np.asarray(inputs["W_out"], dtype=np.float32))
    b_out = np.asarray(inputs["b_out"], dtype=np.float32).ravel()

    # edge run boundaries for every 128-atom sub-window of every core
    bounds = []
    for c in range(NCORES):
        for s in range(NSW):
            bounds.append(c * APC + s * SUB)
    bounds.append(NA)
    edges = np.searchsorted(seg_i, np.asarray(bounds, dtype=np.int64))

    # per-sub-window lo/hi (by idx_j half) counts -> global chunk capacities
    nsw_tot = NCORES * NSW
    lo_masks = []
    n_lo = np.zeros(nsw_tot, dtype=np.int64)
    n_hi = np.zeros(nsw_tot, dtype=np.int64)
    for k in range(nsw_tot):
        lo, hi = edges[k], edges[k + 1]
        m = idx_j[lo:hi] < HALF
        lo_masks.append(m)
        n_lo[k] = int(m.sum())
        n_hi[k] = int((hi - lo) - n_lo[k])
    cap_lo = max(1, int(-(-n_lo.max() // P)))
    cap_hi = max(1, int(-(-n_hi.max() // P)))
    cap = cap_lo + cap_hi
    esw = cap * P
    # per-(s,half) static gather sizes: 16-rounded max real count over cores
    n_lo2 = n_lo.reshape(NCORES, NSW)
    n_hi2 = n_hi.reshape(NCORES, NSW)
    n16 = []
    for s in range(NSW):
        n16.append(
            (
                max(16, int(-(-n_lo2[:, s].max() // 16)) * 16),
                max(16, int(-(-n_hi2[:, s].max() // 16)) * 16),
            )
        )
    icols = [n[0] // 16 + n[1] // 16 for n in n16]
    ntot = sum(icols)

    NPBF = ml_dtypes.bfloat16
    iota_t = np.tile(np.arange(P, dtype=np.float32), (P, cap)).astype(NPBF)
    bias_t = np.tile(b_out[None, :], (P, 1)).astype(np.float32)
    xT = np.ascontiguousarray(x.T).astype(NPBF)

    in_maps = []
    for c in range(NCORES):
        wdev = np.zeros((NSW, P, esw), dtype=np.float32)
        segw = np.zeros((P, NSW * cap), dtype=np.float32)
        idx16 = np.zeros((P, ntot), dtype=np.int16)
        for s in range(NSW):
            k = c * NSW + s
            lo, hi = edges[k], edges[k + 1]
            m = lo_masks[k]
            e_idx = idx_j[lo:hi]
            e_seg = (seg_i[lo:hi] - (c * APC + s * SUB)).astype(np.float32)
            e_w = w_ij[lo:hi]
            nl = int(n_lo[k])
            nh = int(n_hi[k])

            wpad = np.zeros((esw, D), dtype=np.float32)
            spad = np.zeros(esw, dtype=np.float32)
            ilo = np.zeros(n16[s][0], dtype=np.int16)
            ihi = np.zeros(n16[s][1], dtype=np.int16)

            wpad[:nl] = e_w[m]
            spad[:nl] = e_seg[m]
            ilo[:nl] = _remap(e_idx[m]).astype(np.int16)
            base = cap_lo * P
            wpad[base : base + nh] = e_w[~m]
            spad[base : base + nh] = e_seg[~m]
            ihi[:nh] = _remap(e_idx[~m] - HALF).astype(np.int16)

            wdev[s] = wpad.reshape(cap, P, D).transpose(1, 0, 2).reshape(P, esw)
            segw[:, s * cap : (s + 1) * cap] = spad.reshape(cap, P).T
            io = sum(icols[:s])
            idx16[:, io : io + n16[s][0] // 16] = _wrap_idx(ilo)
            idx16[:, io + n16[s][0] // 16 : io + icols[s]] = _wrap_idx(ihi)
        in_maps.append(
            {
                "xT": xT,
                "wdev": wdev.astype(NPBF),
                "segw": segw.astype(NPBF),
                "idx16": idx16,
                "iota": iota_t,
                "Win": W_in.astype(NPBF),
                "Wout": W_out.astype(NPBF),
                "bias": bias_t,
            }
        )
    return (cap_lo, cap_hi, n16), in_maps


def kernel(**inputs) -> np.ndarray:
    from concourse.bass_utils import run_bass_kernel_spmd

    plan, in_maps = prepare(inputs)
    nc = build_program(plan)
    nc.finalize()
    res = run_bass_kernel_spmd(nc, in_maps, core_ids=list(range(NCORES)))
    return np.concatenate([r["out"] for r in res.results], axis=0)

